# revision 53
# baseline (speedup 1.0000x reference)
"""Trainium2 Bass kernel for conv-projected multi-head attention.

Reference computation (per batch element b of 8):
  q  = conv1x1(x, Wq)                     # [512, 32, 32]
  kv = conv3x3(x, Wkv, pad=1)             # [1024, 32, 32] -> k, v
  per head h (8 heads, d=64): attn = softmax(q k^T / sqrt(d)); o = attn v
  out = conv1x1(gelu(o), Wout) + bout     # [256, 32, 32]

Sharding: data-parallel over batch. Core b computes batch element b
end-to-end; no collectives.

Per-core kernel structure (all matmuls bf16 inputs -> fp32 PSUM):
  - x held in SBUF zero-padded to [256, 34, 34]; each 3x3 tap is a
    shifted view of it.
  - q/k conv weight-stationary, streaming strided views of the padded
    image -> q^T, k^T in natural [ch, pix] layout.
  - v conv x-stationary over contiguous shifted copies (stationary APs
    must be single-free-dim) -> v lands already transposed [pix, ch],
    written into vaug[j, h, 65] whose 65th column is ones.
  - dots computed transposed: e^T[j, i] = exp(scale * k q^T), two heads
    packed per matmul via PE row-tiling (contraction d=64), attention
    scale folded into the exp activation, no max-subtraction (logits
    are small enough for fp32 exp).
  - attn@v: outT[65, 512] = vaug^T e^T accumulated over j chunks; row 64
    is the softmax denominator (thanks to the ones column).
  - 1/s: sums spread across partitions via a small DMA, exact DVE
    reciprocal, gathered back and broadcast across partitions with a
    K=1 f32r ones-matmul; applied to the drained output rows on DVE.
    (Custom DVE/GPSIMD ops are avoided: they mis-execute on this HW path.)
  - exact gelu on ScalarE, then 1x1 Wout conv + bias, DMA out.
Software pipelining: dots/exp for the first heads overlap the v conv
(ACT is otherwise idle during convs); dots then run ~3 heads ahead of
attn@v; the output projection chases the gelus g-major.
"""

import os
import sys
from contextlib import ExitStack

import numpy as np

sys.path.insert(0, "/opt/trn_rl_repo")

import ml_dtypes  # noqa: E402
import concourse.bass as bass  # noqa: E402
import concourse.tile as tile  # noqa: E402
from concourse import bacc, mybir  # noqa: E402
from concourse.bass_utils import run_bass_kernel_spmd  # noqa: E402

BF16 = ml_dtypes.bfloat16

B, C, H, W = 8, 256, 32, 32
HEADS, D = 8, 64
INNER = HEADS * D  # 512
N = H * W  # 1024
SCALE = D ** -0.5
HP, WP = H + 2, W + 2  # padded image
NCORES = 8

dt = mybir.dt


def emit(tc, ins, out_ap, _debug=False):
    """Emit the per-core kernel. ins: dict name->AP, out_ap: [256, 1024] f32."""
    nc = tc.nc
    ctx = tc._emit_ctx  # ExitStack owned by caller

    consts = ctx.enter_context(tc.tile_pool(name="consts", bufs=1))

    # weight loads: the q tap (tiny) first so the q conv starts early,
    # then x, then per-128-channel chunks of the k taps
    wqk_sb = consts.tile([128, 20, 512], dt.bfloat16, name="wqk_sb")
    wqk_v = ins["wqk"].rearrange("p (c t m) -> p c t m", c=2, t=10, m=512)
    for c2 in range(2):
        nc.sync.dma_start(wqk_sb[:, c2 * 10:c2 * 10 + 1, :],
                          wqk_v[:, c2, 0:1])
    xp_sb = consts.tile([128, 2, HP * WP], dt.bfloat16, name="xp_sb")
    xp_v = ins["xp"].rearrange("p (c n) -> p c n", c=2, n=HP * WP)
    for c2 in range(2):
        nc.sync.dma_start(xp_sb[:, c2:c2 + 1, :], xp_v[:, c2:c2 + 1, :])
    for c2 in range(2):
        nc.sync.dma_start(wqk_sb[:, c2 * 10 + 1:(c2 + 1) * 10, :],
                          wqk_v[:, c2, 1:10])

    # padded image view: [128, c2, 34, 34]
    xv = xp_sb.rearrange("p c (h w) -> p c h w", h=HP, w=WP)

    # contiguous shifted copies, one per 3x3 tap: xs[t][c2] = [128, 1024].
    # Only the v conv needs these (its stationary operand must have a
    # single free dim); the q/k convs stream strided views of xp_sb.
    xs = [[consts.tile([128, N], dt.bfloat16, name=f"xs{t}_{c2}")
           for c2 in range(2)] for t in range(9)]
    for t in range(9):
        ky, kx = t // 3, t % 3
        for c2 in range(2):
            nc.sync.dma_start(
                xs[t][c2].rearrange("p (h w) -> p h w", h=H, w=W),
                xv[:, c2, ky: ky + 32, kx: kx + 32])

    wv_sb = consts.tile([128, 18, 512], dt.bfloat16, name="wv_sb")
    wv_v = ins["wv"].rearrange("p (c t m) -> p c t m", c=2, t=9, m=512)
    for c2 in range(2):
        nc.sync.dma_start(wv_sb[:, c2 * 9:(c2 + 1) * 9, :], wv_v[:, c2])
    wo_sb = consts.tile([128, 4, 256], dt.bfloat16, name="wo_sb")
    nc.sync.dma_start(wo_sb, ins["wo"])
    bias_sb = consts.tile([128, 2], dt.float32, name="bias_sb")
    nc.sync.dma_start(bias_sb, ins["bias"])

    ones_f = consts.tile([65, 64], dt.float32, name="ones_f")
    nc.vector.memset(ones_f, 1.0)
    ones65 = consts.tile([65, 64], dt.float32r, name="ones65")
    nc.vector.tensor_copy(ones65, ones_f)

    # persistent conv outputs (bf16, [ch_chunk 128, 1024 pix])
    q_sb = [consts.tile([128, N], dt.bfloat16, name=f"q_sb{m}") for m in range(4)]
    k_sb = [consts.tile([128, N], dt.bfloat16, name=f"k_sb{m}") for m in range(4)]
    # vaug[jc]: [128 pix, head, 64 v + 1 ones]
    va_sb = [consts.tile([128, HEADS, 65], dt.bfloat16, name=f"va{j}")
             for j in range(8)]
    # unscaled-then-scaled attention output, f32 [128 (2 heads' d), 1024 q]
    og_sb = [consts.tile([128, N], dt.float32, name=f"og{g}") for g in range(4)]
    gg_sb = [consts.tile([128, N], dt.bfloat16, name=f"gg{g}") for g in range(4)]
    out_sb = [consts.tile([128, N], dt.float32, name=f"osb{c}") for c in range(2)]

    etpool = ctx.enter_context(tc.tile_pool(name="etp", bufs=20))
    rtpool = ctx.enter_context(tc.tile_pool(name="rtp", bufs=3))
    # dots psum pool lives from the early-dots overlap through attention
    epool = ctx.enter_context(tc.tile_pool(name="eps", bufs=2, space="PSUM"))

    dbg_sb = None
    if _debug:
        dbg_sb = consts.tile([65, 2048], dt.float32, name="dbg_sb")
        nc.vector.memset(dbg_sb, -7.0)

    et_tiles = {}  # h -> list of 8 eT tiles

    def dots_head(h):
        g, p = h // 2, h % 2
        ps, pe_ = 64 * p, 64 * p + 64
        et_tiles[h] = []
        for jc in range(8):
            pse = epool.tile([128, N], dt.float32, name="eps", tag="eps")
            for ic in range(2):
                lhsT = k_sb[g][ps:pe_, jc * 128:(jc + 1) * 128]
                rhs = q_sb[g][ps:pe_, ic * 512:(ic + 1) * 512]
                nc.tensor.matmul(pse[:, ic * 512:(ic + 1) * 512], lhsT, rhs,
                                 start=True, stop=True)
            et = etpool.tile([128, N], dt.bfloat16, name="et", tag="et")
            nc.scalar.activation(et, pse, mybir.ActivationFunctionType.Exp,
                                 scale=SCALE)
            et_tiles[h].append(et)

    def attnv_head(h, opool, pbpool):
        g, p = h // 2, h % 2
        ps, pe_ = 64 * p, 64 * p + 64
        for ic in range(2):
            po = opool.tile([65, 512], dt.float32, name="ops", tag="ops")
            for jc in range(8):
                nc.tensor.matmul(po, va_sb[jc][:, h, :],
                                 et_tiles[h][jc][:, ic * 512:(ic + 1) * 512],
                                 start=(jc == 0), stop=(jc == 7))
            # drain po right away (frees the psum slot): row 64 = sums,
            # rows 0-63 = unnormalized output
            st = rtpool.tile([65, 512], dt.float32, name="st", tag="st")
            nc.vector.tensor_copy(st[64:65, :], po[64:65, :])
            ogs = og_sb[g][ps:pe_, ic * 512:(ic + 1) * 512]
            nc.vector.tensor_copy(ogs, po[0:64, :])
            # 1/s off the critical path: spread the 512 sums across
            # partitions via DMA, exact DVE reciprocal there (8 cyc/elem
            # is cheap at 4/lane), gather back to a partition-64 row and
            # broadcast to 64 partitions with a K=1 ones-matmul.
            sp = rtpool.tile([128, 4], dt.float32, name="sp", tag="sp")
            nc.sync.dma_start(sp, st[64:65, :].rearrange("p (a b) -> p a b",
                                                         a=128, b=4))
            rp = rtpool.tile([128, 4], dt.float32, name="rp", tag="rp")
            nc.vector.reciprocal(rp, sp)
            rt = rtpool.tile([65, 512], dt.float32, name="rt", tag="rt")
            # single-partition writes are slow; split the gather across the
            # two DMA paths so the halves run concurrently
            rtv = rt[64:65, :].rearrange("p (a b) -> p a b", a=128, b=4)
            nc.sync.dma_start(rtv[:, 0:64, :], rp[0:64, :])
            nc.gpsimd.dma_start(rtv[:, 64:128, :], rp[64:128, :])
            rtr = rtpool.tile([65, 512], dt.float32r, name="rtr", tag="rtr")
            nc.vector.tensor_copy(rtr[64:65, :], rt[64:65, :])
            pb = pbpool.tile([64, 512], dt.float32, name="bcp", tag="bcp")
            nc.tensor.matmul(pb, ones65[64:65, :], rtr[64:65, :],
                             start=True, stop=True)
            if _debug and h == 0 and ic == 0:
                nc.vector.tensor_copy(dbg_sb[64:65, 0:512], st[64:65, :])
                nc.vector.tensor_copy(dbg_sb[64:65, 512:1024], rt[64:65, :])
                nc.vector.tensor_copy(dbg_sb[64:65, 1024:1536], rtr[64:65, :])
                nc.vector.tensor_copy(dbg_sb[0:64, 1536:2048], pb[:, :])
            nc.vector.tensor_mul(ogs, ogs, pb[:, :])

    with tc.tile_pool(name="cps", bufs=4, space="PSUM") as cpool:
        # ---- q (1 tap) and k (9 taps) convs, weight stationary ----
        # mc 0-3: q chunks (tap list [0]); mc 4-7: k chunks (taps 1..9)
        for mc in range(8):
            is_q = mc < 4
            taps = [0] if is_q else list(range(1, 10))
            mi = mc if is_q else mc - 4
            for nh in range(2):
                pe = cpool.tile([128, 512], dt.float32, name="cps", tag="cps")
                seq = [(t, c2) for t in taps for c2 in range(2)]
                y0 = 16 * nh
                for i, (t, c2) in enumerate(seq):
                    xt = 4 if t == 0 else t - 1  # q uses center tap
                    ky, kx = xt // 3, xt % 3
                    lhsT = wqk_sb[:, c2 * 10 + t, mi * 128:(mi + 1) * 128]
                    rhs = xv[:, c2, ky + y0: ky + y0 + 16, kx: kx + 32]
                    nc.tensor.matmul(pe, lhsT, rhs,
                                     start=(i == 0), stop=(i == len(seq) - 1))
                dest = (q_sb if is_q else k_sb)[mi][:, nh * 512:(nh + 1) * 512]
                nc.vector.tensor_copy(dest, pe)

        # dots for the first heads overlap the v conv (uses the
        # otherwise-idle ACT engine during the conv phase)
        dots_head(0)
        dots_head(1)

        # ---- v conv, x stationary -> transposed output ----
        for jc in range(8):
            pv = cpool.tile([128, 512], dt.float32, name="vps", tag="cps")
            seq = [(t, c2) for t in range(9) for c2 in range(2)]
            for i, (t, c2) in enumerate(seq):
                lhsT = xs[t][c2][:, jc * 128:(jc + 1) * 128]
                rhs = wv_sb[:, c2 * 9 + t, :]
                nc.tensor.matmul(pv, lhsT, rhs,
                                 start=(i == 0), stop=(i == len(seq) - 1))
            nc.vector.memset(va_sb[jc][:, :, 64:65], 1.0)
            nc.vector.tensor_copy(va_sb[jc][:, :, 0:64], pv)

    # ---- attention, software-pipelined: dots runs ~2.5 heads ahead ----
    with tc.tile_pool(name="ops", bufs=2, space="PSUM") as opool, \
         tc.tile_pool(name="pbp", bufs=2, space="PSUM") as pbpool:
        dots_head(2)
        for h in range(HEADS):
            attnv_head(h, opool, pbpool)
            if h + 3 < HEADS:
                dots_head(h + 3)

    # ---- gelu + output projection, g-major so matmuls chase the gelus ----
    with tc.tile_pool(name="fps", bufs=1, space="PSUM") as fpool:
        pf = {}
        for co in range(2):
            for nh in range(2):
                pf[co, nh] = fpool.tile([128, 512], dt.float32,
                                        name=f"pf{co}{nh}", tag=f"pf{co}{nh}")
        for g in range(4):
            for nh in range(2):
                sl = slice(nh * 512, (nh + 1) * 512)
                nc.scalar.activation(gg_sb[g][:, sl], og_sb[g][:, sl],
                                     mybir.ActivationFunctionType.Gelu)
                for co in range(2):
                    nc.tensor.matmul(
                        pf[co, nh], wo_sb[:, g, co * 128:(co + 1) * 128],
                        gg_sb[g][:, sl],
                        start=(g == 0), stop=(g == 3))
        for co in range(2):
            for nh in range(2):
                nc.vector.tensor_scalar_add(
                    out_sb[co][:, nh * 512:(nh + 1) * 512], pf[co, nh],
                    bias_sb[:, co:co + 1])
                nc.sync.dma_start(
                    out_ap[co * 128:(co + 1) * 128, nh * 512:(nh + 1) * 512],
                    out_sb[co][:, nh * 512:(nh + 1) * 512])

    if _debug:
        return {"q_sb": q_sb, "k_sb": k_sb, "va_sb": va_sb, "og_sb": og_sb,
                "gg_sb": gg_sb, "xs": xs, "out_sb": out_sb,
                "dbg_sb": dbg_sb}


def build_nc(repeat=1):
    nc = bacc.Bacc(trn_type="TRN2", target_bir_lowering=False, debug=False)
    ins = {
        "xp": nc.dram_tensor("xp", [128, 2 * HP * WP], dt.bfloat16,
                             kind="ExternalInput").ap(),
        "wqk": nc.dram_tensor("wqk", [128, 20 * 512], dt.bfloat16,
                              kind="ExternalInput").ap(),
        "wv": nc.dram_tensor("wv", [128, 18 * 512], dt.bfloat16,
                             kind="ExternalInput").ap(),
        "wo": nc.dram_tensor("wo", [128, 4 * 256], dt.bfloat16,
                             kind="ExternalInput").ap(),
        "bias": nc.dram_tensor("bias", [128, 2], dt.float32,
                               kind="ExternalInput").ap(),
    }
    out_ap = nc.dram_tensor("out", [256, N], dt.float32,
                            kind="ExternalOutput").ap()
    with tile.TileContext(nc) as tc:
        for _ in range(repeat):
            with ExitStack() as ctx:
                tc._emit_ctx = ctx
                emit(tc, ins, out_ap)
    nc.compile()
    return nc


def pack_weights(Wq, Wkv, Wout, bout):
    """Host-side packing of weights into the DRAM layouts the kernel expects."""
    # q tap + 9 k taps, each transposed to [c_in, c_out]: [10, 256, 512]
    qk = [Wq[:, :, 0, 0].T]
    for t in range(9):
        qk.append(Wkv[0:INNER, :, t // 3, t % 3].T)
    qk = np.stack(qk)  # [10, 256, 512]
    wqk = (qk.transpose(1, 0, 2)        # [256, 10, 512]
             .reshape(2, 128, 10, 512)
             .transpose(1, 0, 2, 3)     # [128, 2, 10, 512]
             .reshape(128, 20 * 512).astype(BF16))
    vv = np.stack([Wkv[INNER:, :, t // 3, t % 3].T for t in range(9)])
    wv = (vv.transpose(1, 0, 2)
            .reshape(2, 128, 9, 512)
            .transpose(1, 0, 2, 3)
            .reshape(128, 18 * 512).astype(BF16))
    wo = (Wout[:, :, 0, 0].T            # [512, 256]
          .reshape(4, 128, 256)
          .transpose(1, 0, 2)
          .reshape(128, 4 * 256).astype(BF16))
    bias = np.ascontiguousarray(bout.reshape(2, 128).T).astype(np.float32)
    return wqk, wv, wo, bias


def pack_x(xb):
    """One batch element [256, 32, 32] -> padded [128, 2*34*34] bf16."""
    xpad = np.zeros((C, HP, WP), np.float32)
    xpad[:, 1:33, 1:33] = xb
    return np.ascontiguousarray(
        xpad.reshape(2, 128, HP * WP).transpose(1, 0, 2)
            .reshape(128, 2 * HP * WP)).astype(BF16)


_compiled = {}


def kernel(x, Wq, Wkv, Wout, bout, _trace=False, _tmpdir=None):
    x = np.asarray(x, np.float32)
    Wq = np.asarray(Wq, np.float32)
    Wkv = np.asarray(Wkv, np.float32)
    Wout = np.asarray(Wout, np.float32)
    bout = np.asarray(bout, np.float32)

    if "nc" not in _compiled:
        _compiled["nc"] = build_nc()
    nc = _compiled["nc"]

    wqk, wv, wo, bias = pack_weights(Wq, Wkv, Wout, bout)
    in_maps = []
    for b in range(NCORES):
        in_maps.append({
            "xp": pack_x(x[b]),
            "wqk": wqk, "wv": wv, "wo": wo, "bias": bias,
        })

    res = run_bass_kernel_spmd(nc, in_maps, core_ids=list(range(NCORES)),
                               trace=_trace, tmpdir=_tmpdir)
    outs = [res.results[b]["out"].reshape(C, H, W) for b in range(NCORES)]
    full = np.stack(outs).astype(np.float32)
    if _trace:
        return full, res
    return full


# revision 54
# speedup vs baseline: 1.0366x; 1.0366x over previous
"""Trainium2 Bass kernel for conv-projected multi-head attention.

Reference computation (per batch element b of 8):
  q  = conv1x1(x, Wq)                     # [512, 32, 32]
  kv = conv3x3(x, Wkv, pad=1)             # [1024, 32, 32] -> k, v
  per head h (8 heads, d=64): attn = softmax(q k^T / sqrt(d)); o = attn v
  out = conv1x1(gelu(o), Wout) + bout     # [256, 32, 32]

Sharding: data-parallel over batch. Core b computes batch element b
end-to-end; no collectives.

Per-core kernel structure (all matmuls bf16 inputs -> fp32 PSUM):
  - x held in SBUF zero-padded to [256, 34, 34]; each 3x3 tap is a
    shifted view of it.
  - q/k conv weight-stationary, streaming strided views of the padded
    image -> q^T, k^T in natural [ch, pix] layout.
  - v conv x-stationary over contiguous shifted copies (stationary APs
    must be single-free-dim) -> v lands already transposed [pix, ch],
    written into vaug[j, h, 65] whose 65th column is ones.
  - dots computed transposed: e^T[j, i] = exp(scale * k q^T), two heads
    packed per matmul via PE row-tiling (contraction d=64), attention
    scale folded into the exp activation, no max-subtraction (logits
    are small enough for fp32 exp).
  - attn@v: outT[65, 512] = vaug^T e^T accumulated over j chunks; row 64
    is the softmax denominator (thanks to the ones column).
  - 1/s: sums spread across partitions via a small DMA, exact DVE
    reciprocal, gathered back and broadcast across partitions with a
    K=1 f32r ones-matmul; applied to the drained output rows on DVE.
    (Custom DVE/GPSIMD ops are avoided: they mis-execute on this HW path.)
  - exact gelu on ScalarE, then 1x1 Wout conv + bias, DMA out.
Software pipelining: dots/exp for the first heads overlap the v conv
(ACT is otherwise idle during convs); dots then run ~3 heads ahead of
attn@v; the output projection chases the gelus g-major.
"""

import os
import sys
from contextlib import ExitStack

import numpy as np

sys.path.insert(0, "/opt/trn_rl_repo")

import ml_dtypes  # noqa: E402
import concourse.bass as bass  # noqa: E402
import concourse.tile as tile  # noqa: E402
from concourse import bacc, mybir  # noqa: E402
from concourse.bass_utils import run_bass_kernel_spmd  # noqa: E402

BF16 = ml_dtypes.bfloat16

B, C, H, W = 8, 256, 32, 32
HEADS, D = 8, 64
INNER = HEADS * D  # 512
N = H * W  # 1024
SCALE = D ** -0.5
HP, WP = H + 2, W + 2  # padded image
NCORES = 8

dt = mybir.dt


def emit(tc, ins, out_ap, _debug=False):
    """Emit the per-core kernel. ins: dict name->AP, out_ap: [256, 1024] f32."""
    nc = tc.nc
    ctx = tc._emit_ctx  # ExitStack owned by caller

    consts = ctx.enter_context(tc.tile_pool(name="consts", bufs=1))

    # weight loads: the q tap (tiny) first so the q conv starts early,
    # then x, then per-128-channel chunks of the k taps
    wqk_sb = consts.tile([128, 20, 512], dt.bfloat16, name="wqk_sb")
    wqk_v = ins["wqk"].rearrange("p (c t m) -> p c t m", c=2, t=10, m=512)
    for c2 in range(2):
        nc.sync.dma_start(wqk_sb[:, c2 * 10:c2 * 10 + 1, :],
                          wqk_v[:, c2, 0:1])
    xp_sb = consts.tile([128, 2, HP * WP], dt.bfloat16, name="xp_sb")
    xp_v = ins["xp"].rearrange("p (c n) -> p c n", c=2, n=HP * WP)
    for c2 in range(2):
        nc.sync.dma_start(xp_sb[:, c2:c2 + 1, :], xp_v[:, c2:c2 + 1, :])
    for c2 in range(2):
        eng = nc.sync if c2 == 0 else nc.gpsimd
        eng.dma_start(wqk_sb[:, c2 * 10 + 1:(c2 + 1) * 10, :],
                      wqk_v[:, c2, 1:10])

    # padded image view: [128, c2, 34, 34]
    xv = xp_sb.rearrange("p c (h w) -> p c h w", h=HP, w=WP)

    # contiguous shifted copies, one per 3x3 tap: xs[t][c2] = [128, 1024].
    # Only the v conv needs these (its stationary operand must have a
    # single free dim); the q/k convs stream strided views of xp_sb.
    xs = [[consts.tile([128, N], dt.bfloat16, name=f"xs{t}_{c2}")
           for c2 in range(2)] for t in range(9)]
    for t in range(9):
        ky, kx = t // 3, t % 3
        for c2 in range(2):
            eng = nc.sync if (t + c2) % 2 == 0 else nc.gpsimd
            eng.dma_start(
                xs[t][c2].rearrange("p (h w) -> p h w", h=H, w=W),
                xv[:, c2, ky: ky + 32, kx: kx + 32])

    wv_sb = consts.tile([128, 18, 512], dt.bfloat16, name="wv_sb")
    wv_v = ins["wv"].rearrange("p (c t m) -> p c t m", c=2, t=9, m=512)
    for c2 in range(2):
        eng = nc.sync if c2 == 0 else nc.gpsimd
        eng.dma_start(wv_sb[:, c2 * 9:(c2 + 1) * 9, :], wv_v[:, c2])
    wo_sb = consts.tile([128, 4, 256], dt.bfloat16, name="wo_sb")
    nc.sync.dma_start(wo_sb, ins["wo"])
    bias_sb = consts.tile([128, 2], dt.float32, name="bias_sb")
    nc.sync.dma_start(bias_sb, ins["bias"])

    ones_f = consts.tile([65, 64], dt.float32, name="ones_f")
    nc.vector.memset(ones_f, 1.0)
    ones65 = consts.tile([65, 64], dt.float32r, name="ones65")
    nc.vector.tensor_copy(ones65, ones_f)

    # persistent conv outputs (bf16, [ch_chunk 128, 1024 pix])
    q_sb = [consts.tile([128, N], dt.bfloat16, name=f"q_sb{m}") for m in range(4)]
    k_sb = [consts.tile([128, N], dt.bfloat16, name=f"k_sb{m}") for m in range(4)]
    # vaug[jc]: [128 pix, head, 64 v + 1 ones]
    va_sb = [consts.tile([128, HEADS, 65], dt.bfloat16, name=f"va{j}")
             for j in range(8)]
    # unscaled-then-scaled attention output, f32 [128 (2 heads' d), 1024 q]
    og_sb = [consts.tile([128, N], dt.float32, name=f"og{g}") for g in range(4)]
    gg_sb = [consts.tile([128, N], dt.bfloat16, name=f"gg{g}") for g in range(4)]
    out_sb = [consts.tile([128, N], dt.float32, name=f"osb{c}") for c in range(2)]

    etpool = ctx.enter_context(tc.tile_pool(name="etp", bufs=20))
    rtpool = ctx.enter_context(tc.tile_pool(name="rtp", bufs=3))
    # dots psum pool lives from the early-dots overlap through attention
    epool = ctx.enter_context(tc.tile_pool(name="eps", bufs=2, space="PSUM"))

    dbg_sb = None
    if _debug:
        dbg_sb = consts.tile([65, 2048], dt.float32, name="dbg_sb")
        nc.vector.memset(dbg_sb, -7.0)

    et_tiles = {}  # h -> list of 8 eT tiles

    def dots_head(h):
        g, p = h // 2, h % 2
        ps, pe_ = 64 * p, 64 * p + 64
        et_tiles[h] = []
        for jc in range(8):
            pse = epool.tile([128, N], dt.float32, name="eps", tag="eps")
            for ic in range(2):
                lhsT = k_sb[g][ps:pe_, jc * 128:(jc + 1) * 128]
                rhs = q_sb[g][ps:pe_, ic * 512:(ic + 1) * 512]
                nc.tensor.matmul(pse[:, ic * 512:(ic + 1) * 512], lhsT, rhs,
                                 start=True, stop=True)
            et = etpool.tile([128, N], dt.bfloat16, name="et", tag="et")
            nc.scalar.activation(et, pse, mybir.ActivationFunctionType.Exp,
                                 scale=SCALE)
            et_tiles[h].append(et)

    def attnv_head(h, opool, pbpool):
        g, p = h // 2, h % 2
        ps, pe_ = 64 * p, 64 * p + 64
        for ic in range(2):
            po = opool.tile([65, 512], dt.float32, name="ops", tag="ops")
            for jc in range(8):
                nc.tensor.matmul(po, va_sb[jc][:, h, :],
                                 et_tiles[h][jc][:, ic * 512:(ic + 1) * 512],
                                 start=(jc == 0), stop=(jc == 7))
            # drain po right away (frees the psum slot): row 64 = sums,
            # rows 0-63 = unnormalized output
            st = rtpool.tile([65, 512], dt.float32, name="st", tag="st")
            nc.vector.tensor_copy(st[64:65, :], po[64:65, :])
            ogs = og_sb[g][ps:pe_, ic * 512:(ic + 1) * 512]
            nc.vector.tensor_copy(ogs, po[0:64, :])
            # 1/s off the critical path: spread the 512 sums across
            # partitions via DMA, exact DVE reciprocal there (8 cyc/elem
            # is cheap at 4/lane), gather back to a partition-64 row and
            # broadcast to 64 partitions with a K=1 ones-matmul.
            sp = rtpool.tile([128, 4], dt.float32, name="sp", tag="sp")
            nc.sync.dma_start(sp, st[64:65, :].rearrange("p (a b) -> p a b",
                                                         a=128, b=4))
            rp = rtpool.tile([128, 4], dt.float32, name="rp", tag="rp")
            nc.vector.reciprocal(rp, sp)
            rt = rtpool.tile([65, 512], dt.float32, name="rt", tag="rt")
            # single-partition writes are slow; split the gather across the
            # two DMA paths so the halves run concurrently
            rtv = rt[64:65, :].rearrange("p (a b) -> p a b", a=128, b=4)
            nc.sync.dma_start(rtv[:, 0:64, :], rp[0:64, :])
            nc.gpsimd.dma_start(rtv[:, 64:128, :], rp[64:128, :])
            rtr = rtpool.tile([65, 512], dt.float32r, name="rtr", tag="rtr")
            nc.vector.tensor_copy(rtr[64:65, :], rt[64:65, :])
            pb = pbpool.tile([64, 512], dt.float32, name="bcp", tag="bcp")
            nc.tensor.matmul(pb, ones65[64:65, :], rtr[64:65, :],
                             start=True, stop=True)
            if _debug and h == 0 and ic == 0:
                nc.vector.tensor_copy(dbg_sb[64:65, 0:512], st[64:65, :])
                nc.vector.tensor_copy(dbg_sb[64:65, 512:1024], rt[64:65, :])
                nc.vector.tensor_copy(dbg_sb[64:65, 1024:1536], rtr[64:65, :])
                nc.vector.tensor_copy(dbg_sb[0:64, 1536:2048], pb[:, :])
            nc.vector.tensor_mul(ogs, ogs, pb[:, :])

    with tc.tile_pool(name="cps", bufs=4, space="PSUM") as cpool:
        # ---- q (1 tap) and k (9 taps) convs, weight stationary ----
        # mc 0-3: q chunks (tap list [0]); mc 4-7: k chunks (taps 1..9)
        for mc in range(8):
            is_q = mc < 4
            taps = [0] if is_q else list(range(1, 10))
            mi = mc if is_q else mc - 4
            for nh in range(2):
                pe = cpool.tile([128, 512], dt.float32, name="cps", tag="cps")
                seq = [(t, c2) for t in taps for c2 in range(2)]
                y0 = 16 * nh
                for i, (t, c2) in enumerate(seq):
                    xt = 4 if t == 0 else t - 1  # q uses center tap
                    ky, kx = xt // 3, xt % 3
                    lhsT = wqk_sb[:, c2 * 10 + t, mi * 128:(mi + 1) * 128]
                    rhs = xv[:, c2, ky + y0: ky + y0 + 16, kx: kx + 32]
                    nc.tensor.matmul(pe, lhsT, rhs,
                                     start=(i == 0), stop=(i == len(seq) - 1))
                dest = (q_sb if is_q else k_sb)[mi][:, nh * 512:(nh + 1) * 512]
                nc.vector.tensor_copy(dest, pe)

        # dots for the first heads overlap the v conv (uses the
        # otherwise-idle ACT engine during the conv phase)
        dots_head(0)
        dots_head(1)

        # ---- v conv, x stationary -> transposed output ----
        for jc in range(8):
            pv = cpool.tile([128, 512], dt.float32, name="vps", tag="cps")
            seq = [(t, c2) for t in range(9) for c2 in range(2)]
            for i, (t, c2) in enumerate(seq):
                lhsT = xs[t][c2][:, jc * 128:(jc + 1) * 128]
                rhs = wv_sb[:, c2 * 9 + t, :]
                nc.tensor.matmul(pv, lhsT, rhs,
                                 start=(i == 0), stop=(i == len(seq) - 1))
            nc.vector.memset(va_sb[jc][:, :, 64:65], 1.0)
            nc.vector.tensor_copy(va_sb[jc][:, :, 0:64], pv)

    # ---- attention, software-pipelined: dots runs ~2.5 heads ahead ----
    with tc.tile_pool(name="ops", bufs=2, space="PSUM") as opool, \
         tc.tile_pool(name="pbp", bufs=2, space="PSUM") as pbpool:
        dots_head(2)
        for h in range(HEADS):
            attnv_head(h, opool, pbpool)
            if h + 3 < HEADS:
                dots_head(h + 3)

    # ---- gelu + output projection, g-major so matmuls chase the gelus ----
    with tc.tile_pool(name="fps", bufs=1, space="PSUM") as fpool:
        pf = {}
        for co in range(2):
            for nh in range(2):
                pf[co, nh] = fpool.tile([128, 512], dt.float32,
                                        name=f"pf{co}{nh}", tag=f"pf{co}{nh}")
        for g in range(4):
            for nh in range(2):
                sl = slice(nh * 512, (nh + 1) * 512)
                nc.scalar.activation(gg_sb[g][:, sl], og_sb[g][:, sl],
                                     mybir.ActivationFunctionType.Gelu)
                for co in range(2):
                    nc.tensor.matmul(
                        pf[co, nh], wo_sb[:, g, co * 128:(co + 1) * 128],
                        gg_sb[g][:, sl],
                        start=(g == 0), stop=(g == 3))
        for co in range(2):
            for nh in range(2):
                nc.vector.tensor_scalar_add(
                    out_sb[co][:, nh * 512:(nh + 1) * 512], pf[co, nh],
                    bias_sb[:, co:co + 1])
                nc.sync.dma_start(
                    out_ap[co * 128:(co + 1) * 128, nh * 512:(nh + 1) * 512],
                    out_sb[co][:, nh * 512:(nh + 1) * 512])

    if _debug:
        return {"q_sb": q_sb, "k_sb": k_sb, "va_sb": va_sb, "og_sb": og_sb,
                "gg_sb": gg_sb, "xs": xs, "out_sb": out_sb,
                "dbg_sb": dbg_sb}


def build_nc(repeat=1):
    nc = bacc.Bacc(trn_type="TRN2", target_bir_lowering=False, debug=False)
    ins = {
        "xp": nc.dram_tensor("xp", [128, 2 * HP * WP], dt.bfloat16,
                             kind="ExternalInput").ap(),
        "wqk": nc.dram_tensor("wqk", [128, 20 * 512], dt.bfloat16,
                              kind="ExternalInput").ap(),
        "wv": nc.dram_tensor("wv", [128, 18 * 512], dt.bfloat16,
                             kind="ExternalInput").ap(),
        "wo": nc.dram_tensor("wo", [128, 4 * 256], dt.bfloat16,
                             kind="ExternalInput").ap(),
        "bias": nc.dram_tensor("bias", [128, 2], dt.float32,
                               kind="ExternalInput").ap(),
    }
    out_ap = nc.dram_tensor("out", [256, N], dt.float32,
                            kind="ExternalOutput").ap()
    with tile.TileContext(nc) as tc:
        for _ in range(repeat):
            with ExitStack() as ctx:
                tc._emit_ctx = ctx
                emit(tc, ins, out_ap)
    nc.compile()
    return nc


def pack_weights(Wq, Wkv, Wout, bout):
    """Host-side packing of weights into the DRAM layouts the kernel expects."""
    # q tap + 9 k taps, each transposed to [c_in, c_out]: [10, 256, 512]
    qk = [Wq[:, :, 0, 0].T]
    for t in range(9):
        qk.append(Wkv[0:INNER, :, t // 3, t % 3].T)
    qk = np.stack(qk)  # [10, 256, 512]
    wqk = (qk.transpose(1, 0, 2)        # [256, 10, 512]
             .reshape(2, 128, 10, 512)
             .transpose(1, 0, 2, 3)     # [128, 2, 10, 512]
             .reshape(128, 20 * 512).astype(BF16))
    vv = np.stack([Wkv[INNER:, :, t // 3, t % 3].T for t in range(9)])
    wv = (vv.transpose(1, 0, 2)
            .reshape(2, 128, 9, 512)
            .transpose(1, 0, 2, 3)
            .reshape(128, 18 * 512).astype(BF16))
    wo = (Wout[:, :, 0, 0].T            # [512, 256]
          .reshape(4, 128, 256)
          .transpose(1, 0, 2)
          .reshape(128, 4 * 256).astype(BF16))
    bias = np.ascontiguousarray(bout.reshape(2, 128).T).astype(np.float32)
    return wqk, wv, wo, bias


def pack_x(xb):
    """One batch element [256, 32, 32] -> padded [128, 2*34*34] bf16."""
    xpad = np.zeros((C, HP, WP), np.float32)
    xpad[:, 1:33, 1:33] = xb
    return np.ascontiguousarray(
        xpad.reshape(2, 128, HP * WP).transpose(1, 0, 2)
            .reshape(128, 2 * HP * WP)).astype(BF16)


_compiled = {}


def kernel(x, Wq, Wkv, Wout, bout, _trace=False, _tmpdir=None):
    x = np.asarray(x, np.float32)
    Wq = np.asarray(Wq, np.float32)
    Wkv = np.asarray(Wkv, np.float32)
    Wout = np.asarray(Wout, np.float32)
    bout = np.asarray(bout, np.float32)

    if "nc" not in _compiled:
        _compiled["nc"] = build_nc()
    nc = _compiled["nc"]

    wqk, wv, wo, bias = pack_weights(Wq, Wkv, Wout, bout)
    in_maps = []
    for b in range(NCORES):
        in_maps.append({
            "xp": pack_x(x[b]),
            "wqk": wqk, "wv": wv, "wo": wo, "bias": bias,
        })

    res = run_bass_kernel_spmd(nc, in_maps, core_ids=list(range(NCORES)),
                               trace=_trace, tmpdir=_tmpdir)
    outs = [res.results[b]["out"].reshape(C, H, W) for b in range(NCORES)]
    full = np.stack(outs).astype(np.float32)
    if _trace:
        return full, res
    return full
